# revision 27
# baseline (speedup 1.0000x reference)
"""AugLag bit-decomposed linear layer on 8 Trainium2 NeuronCores.

Computes out = x @ w.T + b where w = sum_k(w_twos[..., k] * base[k]) * step.

Sharding: tensor-parallel over output features. Each of the 8 cores gets a
[512, 4096, 8] slice of w_twos (passed bit-plane-major as [8, 4096, 512]),
the full x (pre-transposed on host to [4096, 8192] so the contraction dim
lands on SBUF partitions), and its bias slice. Each core reconstructs its
w.T shard on VectorE (8 fused multiply-add passes) and runs the GEMM on
TensorE in float32r (full-rate fp32, moving free dim 512). The *step scale
and +bias are folded into the PSUM->SBUF drain.
"""

import os

import numpy as np

import concourse.mybir as mybir
import concourse.tile as tile
from concourse import bacc
from concourse.bass_utils import run_bass_kernel_spmd

N_CORES = 8
N_TOK = 8192
IN_F = 4096
OUT_F = 4096
N_BITS = 8
STEP_SIZE = 0.0078125
OF_SH = OUT_F // N_CORES  # 512 output features per core

P = 128


def build_program(base_vals, n_tok=N_TOK, in_f=IN_F, of_sh=OF_SH, n_bits=N_BITS,
                  step=STEP_SIZE, n_repeat=1):
    """Build the per-core Bass program (SPMD: same program on all cores).

    n_repeat > 1 emits the whole kernel body multiple times (benchmarking
    aid: isolates device exec time from host dispatch overhead).
    """
    f32 = mybir.dt.float32
    f32r = mybir.dt.float32r
    KB = in_f // P      # contraction blocks of 128
    TT = n_tok // P     # output row tiles of 128

    nc = bacc.Bacc(None, target_bir_lowering=False, debug=False)
    # xT is declared float32r (same bit layout as f32): the PE consumes it
    # as the fp32r stationary operand and rounds on read; the BIR verifier
    # accepts ExternalInput-sourced fp32r without a producer rounding op.
    xT = nc.declare_dram_parameter("xT", [in_f, n_tok], f32r, isOutput=False)
    w2p = nc.declare_dram_parameter("w2p", [n_bits, in_f, of_sh], f32,
                                    isOutput=False)
    bias = nc.declare_dram_parameter("bias", [P, of_sh], f32, isOutput=False)
    out = nc.declare_dram_parameter("out", [n_tok, of_sh], f32, isOutput=True)

    with tile.TileContext(nc) as tc:
        with (
            tc.tile_pool(name="wt", bufs=1) as wtp,
            tc.tile_pool(name="stage", bufs=2) as stp,
            tc.tile_pool(name="xc", bufs=4) as xcp,
            tc.tile_pool(name="ob", bufs=3) as obp,
            tc.tile_pool(name="cst", bufs=1) as cst,
            tc.tile_pool(name="ps", bufs=8, space="PSUM") as psp,
        ):
            bias_t = cst.tile([P, of_sh], f32)
            nc.sync.dma_start(bias_t[:], bias[:])

            for _rep in range(n_repeat):
                _emit_body(nc, tc, xT, w2p, out, bias_t, base_vals, step,
                           KB, TT, n_bits, of_sh, wtp, stp, xcp, obp, psp,
                           f32, f32r)

    nc.compile()
    return nc


def _emit_body(nc, tc, xT, w2p, out, bias_t, base_vals, step, KB, TT, n_bits,
               of_sh, wtp, stp, xcp, obp, psp, f32, f32r):
    # Phase A: reconstruct w.T shard, one [128, of_sh] fp32r tile per
    # contraction block, kept resident in SBUF (unscaled by step).
    T_CHUNK = 1024 if (TT * P) % 1024 == 0 else P
    TS = T_CHUNK // P                 # token subtiles per chunk
    TC = (TT * P) // T_CHUNK          # number of token chunks
    KBQ = 4 if (KB % 4 == 0 and T_CHUNK > P) else KB  # kb blocks per x tile
    NQ = KB // KBQ
    OB_TS = min(TS, 4)                # token subtiles per output write
    xTr = xT.rearrange("(kb p) t -> p kb t", p=P)

    def load_xq(tcc, q):
        xq = xcp.tile([P, KBQ, T_CHUNK], f32r, tag="xq", name=f"xq{tcc}_{q}")
        tsl = slice(tcc * T_CHUNK, (tcc + 1) * T_CHUNK)
        nc.sync.dma_start(xq[:], xTr[:, q * KBQ:(q + 1) * KBQ, tsl])
        return xq

    # Phase A: reconstruct w.T. Chunk 0's x loads are interleaved into the
    # same HWDGE FIFO so the PE can start as soon as wt tiles land, instead
    # of idling until all of w2 has streamed in.
    wts = []
    pre_xq = {}
    for kb in range(KB):
        st = stp.tile([P, n_bits, of_sh], f32, tag="stage")
        nc.sync.dma_start(
            st[:],
            w2p[:, kb * P:(kb + 1) * P, :].rearrange("k p o -> p k o"),
        )
        # Accumulate bit planes in f32; the final pass writes the
        # fp32r tile the matmul consumes (producer-side rounding,
        # required by the BIR verifier for fp32r matmul inputs).
        acc = stp.tile([P, of_sh], f32, tag="acc")
        nc.vector.tensor_scalar_mul(acc[:], st[:, 0, :], float(base_vals[0]))
        for k in range(1, n_bits - 1):
            nc.vector.scalar_tensor_tensor(
                acc[:], st[:, k, :], float(base_vals[k]), acc[:],
                mybir.AluOpType.mult, mybir.AluOpType.add,
            )
        wt = wtp.tile([P, of_sh], f32r, tag=f"wt{kb}")
        nc.vector.scalar_tensor_tensor(
            wt[:], st[:, n_bits - 1, :], float(base_vals[n_bits - 1]),
            acc[:], mybir.AluOpType.mult, mybir.AluOpType.add,
        )
        wts.append(wt)
        # Interleave the first xq-pool-depth quarters of chunk 0 into the
        # stage FIFO (more would stall on pool slots and head-of-line-block
        # the w2 stream).
        npre = min(NQ, 4)
        spread = KB // npre if npre else KB
        if npre and (kb + 1) % spread == 0:
            q = (kb + 1) // spread - 1
            if q < npre:
                pre_xq[q] = load_xq(0, q)

    # Phase B: GEMM. Token chunks of T_CHUNK; x loaded as kb-quarter tiles
    # [128, KBQ, T_CHUNK] whose DMA pattern is many 4*T_CHUNK-byte runs
    # (measured ~379 GB/s at 2KB runs vs 211 GB/s at 512B). PSUM holds one
    # [128, of_sh] accumulator per 128-token subtile; drain applies *step
    # and +bias; output batched into [128, TS, of_sh] writes.
    outr = out.rearrange("(c j p) o -> c p j o", p=P, j=OB_TS)
    for tcc in range(TC):
        pss = [psp.tile([P, of_sh], f32, name=f"ps{j}", tag="ps")
               for j in range(TS)]
        for q in range(NQ):
            if tcc == 0 and q in pre_xq:
                xq = pre_xq[q]
            else:
                xq = load_xq(tcc, q)
            for kbq in range(KBQ):
                kb = q * KBQ + kbq
                for j in range(TS):
                    nc.tensor.matmul(
                        pss[j][:],
                        xq[:, kbq, j * P:(j + 1) * P],
                        wts[kb][:],
                        start=(kb == 0),
                        stop=(kb == KB - 1),
                    )
        for h in range(TS // OB_TS):
            ob = obp.tile([P, OB_TS, of_sh], f32, tag="ob", name=f"ob{h}")
            for j in range(OB_TS):
                nc.vector.scalar_tensor_tensor(
                    ob[:, j, :], pss[h * OB_TS + j][:], float(step), bias_t[:],
                    mybir.AluOpType.mult, mybir.AluOpType.add,
                )
            # SWDGE (gpsimd) path: keeps output writes out of the sync
            # HWDGE FIFO, where they would head-of-line-block the next
            # chunks' x prefetches while waiting on this chunk's drain.
            nc.gpsimd.dma_start(outr[tcc * (TS // OB_TS) + h], ob[:])


_program_cache = {}


def _get_program(base_vals):
    key = tuple(base_vals)
    if key not in _program_cache:
        _program_cache[key] = build_program(base_vals)
    return _program_cache[key]


def prep_in_maps(x, w_twos, b):
    """Host-side layout prep (no arithmetic): transpose + shard + replicate."""
    xT = np.ascontiguousarray(x.T)  # [IN_F, N_TOK]
    in_maps = []
    for c in range(N_CORES):
        sl = slice(c * OF_SH, (c + 1) * OF_SH)
        # [OF_SH, IN_F, N_BITS] -> bit-plane-major [N_BITS, IN_F, OF_SH]
        w2p = np.ascontiguousarray(w_twos[sl].transpose(2, 1, 0))
        bias = np.ascontiguousarray(
            np.broadcast_to(b[sl][None, :], (P, OF_SH)))
        in_maps.append({"xT": xT, "w2p": w2p, "bias": bias})
    return in_maps


def kernel(x, w_twos, b, base, **_kwargs):
    x = np.asarray(x, dtype=np.float32)
    w_twos = np.asarray(w_twos, dtype=np.float32)
    b = np.asarray(b, dtype=np.float32)
    base_vals = [float(v) for v in np.asarray(base, dtype=np.float32).reshape(-1)]

    nc = _get_program(base_vals)
    in_maps = prep_in_maps(x, w_twos, b)

    kwargs = {}
    if os.environ.get("KERNEL_TRACE"):
        kwargs["trace"] = True
        if os.environ.get("KERNEL_TRACE_DIR"):
            kwargs["tmpdir"] = os.environ["KERNEL_TRACE_DIR"]
    res = run_bass_kernel_spmd(nc, in_maps, list(range(N_CORES)), **kwargs)
    globals()["last_results"] = res
    out = np.concatenate([res.results[c]["out"] for c in range(N_CORES)],
                         axis=1)
    return out
